# revision 16
# baseline (speedup 1.0000x reference)
"""Trainium2 Bass kernel for a 2-layer LSTM (B=256, T=512, I=64, H=256) + linear head.

Strategy (hardcoded, self-contained):
  - Data-parallel over batch across 8 NeuronCores (32 batch elems per core).
  - Per core, both LSTM layers run step-by-step in a feature-blocked layout:
      gate PSUM tile [128=(hblk4, b32), 256=(gate4, hh2, hl32)]
    produced by col-group-packed matmuls (tile_position=(0, 32*m)) that share
    the small transposed-state stationary hT [k, 32].
  - Input projection x@Wih.T and all biases ride the same PSUM accumulation
    (augmented ones-row trick), so there is no separate projection pass.
  - Elementwise gate math runs at full 128 partitions; a single DVE 32x32
    block-transpose per layer-step turns h back into the next step's
    stationary hT.
  - The two output linear layers have no nonlinearity between them and are
    folded host-side into a single [256, 4] matmul + bias.
  - All weights ship as ONE packed DRAM blob (fp16) split into a weights DMA
    plus 4 x-chunk DMAs so step 0 starts before the full input lands.
  - All matmul operands are fp16 (1 PE cycle/row vs 4 for fp32; fp32 PSUM
    accumulation), and the sigmoid/tanh outputs + gate intermediates are fp16
    for DVE 2x mode; cell state c stays fp32.  Measured 1.59 ms vs 2.11 ms
    for the all-fp32 version, rel err 5.7e-4.
"""

import numpy as np

B, T, I, H, O = 256, 512, 64, 256, 4
NCORES = 8
BS = B // NCORES  # 32

# reference gate order is (i, f, g, o); we reorder to (i, f, o, g) so that the
# sigmoid gates are contiguous (cols 0:192) and tanh(g) is cols 192:256.
GATE_PERM = [0, 1, 3, 2]

# weight blob column offsets (fp32 elements, [128, WB_COLS])
OFF_W0 = 0        # Whh0 perm  [128, 2*1024]
OFF_W1 = 2048     # Whh1 perm  [128, 2*1024]
OFF_WX1 = 4096    # Wih1 perm  [128, 2*1024]
OFF_WX0 = 6144    # Wih0 perm + bias row, rows 0:65, [65, 1024]
OFF_B1 = 7168     # bias1 row, row 0, [1, 1024]
OFF_WF = 8192     # folded head weight [128, 2*4]
OFF_BF = 8200     # folded head bias, row 0, [1, 4]
OFF_XT = 8224     # x transposed + ones row, rows 0:65, [65, t_steps*32]
def _wb_cols(t_steps):
    return OFF_XT + t_steps * BS

_CACHED = {}


def _x2g(Wp):
    """Scale the g-gate columns (m*256+192 .. +256) of a permuted [K, 1024]
    block by 2 so tanh(z_g) can be recovered from sigmoid(2*z_g)."""
    Wp = Wp.copy()
    for m in range(4):
        Wp[:, m * 256 + 192 : m * 256 + 256] *= 2.0
    return Wp


def _perm_cols(Wt):
    """Permute gate columns of [K, 1024] (col j = gate_orig*256 + h) into
    col = m*256 + gate_new*64 + hh*32 + hl, where h = hh*128 + m*32 + hl."""
    K = Wt.shape[0]
    W = Wt.reshape(K, 4, 256)[:, GATE_PERM, :]      # [K, gate, h]
    W = W.reshape(K, 4, 2, 4, 32)                    # [K, gate, hh, m, hl]
    W = W.transpose(0, 3, 1, 2, 4)                   # [K, m, gate, hh, hl]
    return np.ascontiguousarray(W.reshape(K, 1024), dtype=np.float32)


def _build_bass(t_steps=T):
    import concourse.mybir as mybir
    import concourse.tile as tile
    from concourse import bacc
    from contextlib import ExitStack

    f32 = mybir.dt.float32
    f16 = mybir.dt.float16
    AF = mybir.ActivationFunctionType

    nc = bacc.Bacc("TRN2", target_bir_lowering=False)

    wb_cols = _wb_cols(t_steps)
    wb_d = nc.dram_tensor("wb", (128, wb_cols), f16, kind="ExternalInput")
    y_d = nc.dram_tensor("y", (BS, O), f32, kind="ExternalOutput")

    with tile.TileContext(nc) as tc, ExitStack() as ctx:
        const = ctx.enter_context(tc.tile_pool(name="const", bufs=1))
        state = ctx.enter_context(tc.tile_pool(name="state", bufs=1))
        work = ctx.enter_context(tc.tile_pool(name="work", bufs=4))
        hts = ctx.enter_context(tc.tile_pool(name="hts", bufs=4))
        psum = ctx.enter_context(tc.tile_pool(name="psum", bufs=3, space="PSUM"))

        wb = const.tile([128, wb_cols], f16)
        nc.sync.dma_start(wb[:, 0:OFF_XT], wb_d[:, 0:OFF_XT])
        nxt = (wb_cols - OFF_XT) // 4
        for ck in range(4):
            a = OFF_XT + ck * nxt
            b = OFF_XT + (ck + 1) * nxt if ck < 3 else wb_cols
            nc.sync.dma_start(wb[:, a:b], wb_d[:, a:b])

        def xt_ap(t):
            return wb[0:65, OFF_XT + BS * t : OFF_XT + BS * t + BS]

        def w0_ap(kc, m):
            return wb[:, OFF_W0 + 1024 * kc + 256 * m : OFF_W0 + 1024 * kc + 256 * m + 256]

        def w1_ap(kc, m):
            return wb[:, OFF_W1 + 1024 * kc + 256 * m : OFF_W1 + 1024 * kc + 256 * m + 256]

        def wx1_ap(kc, m):
            return wb[:, OFF_WX1 + 1024 * kc + 256 * m : OFF_WX1 + 1024 * kc + 256 * m + 256]

        def wx0_ap(m):
            return wb[0:65, OFF_WX0 + 256 * m : OFF_WX0 + 256 * m + 256]

        def b1_ap(m):
            return wb[0:1, OFF_B1 + 256 * m : OFF_B1 + 256 * m + 256]

        c0 = state.tile([128, 64], f16)
        c1 = state.tile([128, 64], f16)
        nc.vector.memset(c0[:], 0.0)
        nc.vector.memset(c1[:], 0.0)
        hT0 = hts.tile([128, 64], f16, tag="ht0")
        hT1 = hts.tile([128, 64], f16, tag="ht1")
        nc.vector.memset(hT0[:], 0.0)
        nc.vector.memset(hT1[:], 0.0)
        ones_t = const.tile([1, BS], f16)
        nc.vector.memset(ones_t[:], 1.0)
        ones_ap = ones_t[:]

        Alu = mybir.AluOpType

        def elementwise(g, c, tagsuf):
            # g cols [i|f|o|g'] with g' = 2*z_g (weights pre-scaled x2), so one
            # sigmoid covers all 4 blocks and tanh(z_g) = 2*sigmoid(2*z_g)-1.
            sg = work.tile([128, 256], f16, tag="sg" + tagsuf)
            nc.scalar.activation(sg[:], g[:], AF.Sigmoid)
            nc.vector.tensor_mul(c[:], sg[:, 64:128], c[:])
            m1 = work.tile([128, 64], f16, tag="m1" + tagsuf)
            nc.vector.scalar_tensor_tensor(m1[:], sg[:, 192:256], 2.0, sg[:, 0:64],
                                           op0=Alu.mult, op1=Alu.mult)
            nc.vector.tensor_sub(m1[:], m1[:], sg[:, 0:64])
            nc.vector.tensor_add(c[:], c[:], m1[:])
            th = work.tile([128, 64], f16, tag="th" + tagsuf)
            nc.scalar.activation(th[:], c[:], AF.Tanh)
            h = work.tile([128, 64], f16, tag="h" + tagsuf)
            nc.vector.tensor_mul(h[:], sg[:, 128:192], th[:])
            hT = hts.tile([128, 64], f16, tag="ht" + tagsuf)
            nc.vector.transpose(hT[:], h[:])
            return hT

        def step0(t, hT0_prev):
            g = psum.tile([128, 256], f32, tag="g0")
            for m in range(4):
                nc.tensor.matmul(
                    g[32 * m : 32 * m + 32, :], xt_ap(t), wx0_ap(m),
                    start=True, stop=False, tile_position=(0, 32 * m), skip_group_check=True,
                )
            for kc in range(2):
                for m in range(4):
                    nc.tensor.matmul(
                        g[32 * m : 32 * m + 32, :],
                        hT0_prev[:, 32 * kc : 32 * kc + 32], w0_ap(kc, m),
                        start=False, stop=(kc == 1), tile_position=(0, 32 * m), skip_group_check=True,
                    )
            return elementwise(g, c0, "0")

        def step1(hT0_t, hT1_prev):
            g = psum.tile([128, 256], f32, tag="g1")
            for m in range(4):
                nc.tensor.matmul(
                    g[32 * m : 32 * m + 32, :], ones_ap, b1_ap(m),
                    start=True, stop=False, tile_position=(0, 32 * m), skip_group_check=True,
                )
            for si, (src, w_ap) in enumerate(((hT0_t, wx1_ap), (hT1_prev, w1_ap))):
                last_src = w_ap is w1_ap
                for kc in range(2):
                    for m in range(4):
                        nc.tensor.matmul(
                            g[32 * m : 32 * m + 32, :],
                            src[:, 32 * kc : 32 * kc + 32], w_ap(kc, m),
                            start=False,
                            stop=(last_src and kc == 1),
                            tile_position=(0, 32 * m), skip_group_check=True,
                        )
            return elementwise(g, c1, "1")

        hT0_hist = [hT0]
        for t in range(t_steps):
            hT0_new = step0(t, hT0_hist[-1])
            hT0_hist.append(hT0_new)
            # layer 1 lags by one step so the two chains overlap
            if t >= 1:
                hT1 = step1(hT0_hist[-2], hT1)
            if len(hT0_hist) > 3:
                hT0_hist.pop(0)
        hT1 = step1(hT0_hist[-1], hT1)

        yp = psum.tile([BS, O], f32, tag="yh", bufs=1)
        nc.tensor.matmul(yp[:], ones_ap, wb[0:1, OFF_BF : OFF_BF + O], start=True, stop=False)
        nc.tensor.matmul(yp[:], hT1[:, 0:32], wb[:, OFF_WF : OFF_WF + O], start=False, stop=False)
        nc.tensor.matmul(yp[:], hT1[:, 32:64], wb[:, OFF_WF + O : OFF_WF + 2 * O], start=False, stop=True)
        y_sb = work.tile([BS, O], f32, tag="y")
        nc.vector.tensor_copy(y_sb[:], yp[:])
        nc.sync.dma_start(y_d[:], y_sb[:])

    return nc


def _prep_inputs(x, Wih0, Whh0, bih0, bhh0, Wih1, Whh1, bih1, bhh1, W1, b1, W2, b2,
                 t_steps=T):
    x = np.asarray(x, dtype=np.float32)[:, :t_steps, :]
    wb = np.zeros((128, _wb_cols(t_steps)), np.float16)
    wb[:, OFF_W0 : OFF_W0 + 2048] = _x2g(_perm_cols(
        np.asarray(Whh0, np.float32).T)).reshape(2, 128, 1024).transpose(1, 0, 2).reshape(128, 2048)
    wb[:, OFF_W1 : OFF_W1 + 2048] = _x2g(_perm_cols(
        np.asarray(Whh1, np.float32).T)).reshape(2, 128, 1024).transpose(1, 0, 2).reshape(128, 2048)
    wb[:, OFF_WX1 : OFF_WX1 + 2048] = _x2g(_perm_cols(
        np.asarray(Wih1, np.float32).T)).reshape(2, 128, 1024).transpose(1, 0, 2).reshape(128, 2048)
    wb[0:64, OFF_WX0 : OFF_WX0 + 1024] = _x2g(_perm_cols(np.asarray(Wih0, np.float32).T))
    wb[64, OFF_WX0 : OFF_WX0 + 1024] = _x2g(_perm_cols(
        (np.asarray(bih0, np.float32) + np.asarray(bhh0, np.float32))[None, :]))[0]
    wb[0, OFF_B1 : OFF_B1 + 1024] = _x2g(_perm_cols(
        (np.asarray(bih1, np.float32) + np.asarray(bhh1, np.float32))[None, :]))[0]
    Wf = (np.asarray(W1, np.float32).T @ np.asarray(W2, np.float32).T).astype(np.float32)
    wb[:, OFF_WF : OFF_WF + 2 * O] = Wf.reshape(2, 128, O).transpose(1, 0, 2).reshape(128, 2 * O)
    wb[0, OFF_BF : OFF_BF + O] = (
        np.asarray(b1, np.float32) @ np.asarray(W2, np.float32).T + np.asarray(b2, np.float32))

    in_maps = []
    for c in range(NCORES):
        xc = x[c * BS : (c + 1) * BS]                       # [BS, t, I]
        xt = xc.transpose(2, 1, 0).reshape(I, t_steps * BS) # [I, t*BS]
        wbc = wb.copy()
        wbc[0:64, OFF_XT:] = xt
        wbc[64, OFF_XT:] = 1.0
        in_maps.append(dict(wb=wbc))
    return in_maps


def run(t_steps=T, trace=False, **inputs):
    from concourse.bass_utils import run_bass_kernel_spmd

    key = t_steps
    if key not in _CACHED:
        nc_new = _build_bass(t_steps)
        # finalize BEFORE handing to the PJRT path: the bass_exec lowering
        # otherwise finalizes with the partition-id register preamble in a
        # state that miscompiles (walrus "Reg has not been allocated yet")
        nc_new.finalize()
        _CACHED[key] = nc_new
    nc = _CACHED[key]
    in_maps = _prep_inputs(**inputs, t_steps=t_steps)
    res = None
    for attempt in range(4):
        try:
            res = run_bass_kernel_spmd(nc, in_maps, core_ids=list(range(NCORES)),
                                       trace=trace)
            break
        except Exception as e:  # flaky parallel-birverifier race in neuronx-cc
            if attempt == 3:
                raise
            print(f"run attempt {attempt} failed ({type(e).__name__}); retrying")
    assert res is not None
    y = np.concatenate([r["y"] for r in res.results], axis=0)
    return y, res


def kernel(**inputs):
    y, _ = run(t_steps=T, trace=False, **inputs)
    return y



# revision 17
# speedup vs baseline: 1.1279x; 1.1279x over previous
"""Trainium2 Bass kernel for a 2-layer LSTM (B=256, T=512, I=64, H=256) + linear head.

Strategy (hardcoded, self-contained):
  - Data-parallel over batch across 8 NeuronCores (32 batch elems per core).
  - Per core, both LSTM layers run step-by-step in a feature-blocked layout:
      gate PSUM tile [128=(hblk4, b32), 256=(gate4, hh2, hl32)]
    produced by col-group-packed matmuls (tile_position=(0, 32*m)) that share
    the small transposed-state stationary hT [k, 32].
  - Input projection x@Wih.T and all biases ride the same PSUM accumulation
    (augmented ones-row trick), so there is no separate projection pass.
  - Elementwise gate math runs at full 128 partitions; a single DVE 32x32
    block-transpose per layer-step turns h back into the next step's
    stationary hT.
  - The two output linear layers have no nonlinearity between them and are
    folded host-side into a single [256, 4] matmul + bias.
  - All weights ship as ONE packed DRAM blob (fp16) split into a weights DMA
    plus 4 x-chunk DMAs so step 0 starts before the full input lands.
  - All matmul operands are fp16 (1 PE cycle/row vs 4 for fp32; fp32 PSUM
    accumulation), and the sigmoid/tanh outputs + gate intermediates are fp16
    for DVE 2x mode; cell state c stays fp32.  Measured 1.59 ms vs 2.11 ms
    for the all-fp32 version, rel err 5.7e-4.
"""

import numpy as np

B, T, I, H, O = 256, 512, 64, 256, 4
NCORES = 8
BS = B // NCORES  # 32

# reference gate order is (i, f, g, o); we reorder to (i, f, o, g) so that the
# sigmoid gates are contiguous (cols 0:192) and tanh(g) is cols 192:256.
GATE_PERM = [0, 1, 3, 2]

# weight blob column offsets (fp32 elements, [128, WB_COLS])
OFF_W0 = 0        # Whh0 perm  [128, 2*1024]
OFF_W1 = 2048     # Whh1 perm  [128, 2*1024]
OFF_WX1 = 4096    # Wih1 perm  [128, 2*1024]
OFF_WX0 = 6144    # Wih0 perm + bias row, rows 0:65, [65, 1024]
OFF_B1 = 7168     # bias1 row, row 0, [1, 1024]
OFF_WF = 8192     # folded head weight [128, 2*4]
OFF_BF = 8200     # folded head bias, row 0, [1, 4]
OFF_XT = 8224     # x transposed + ones row, rows 0:65, [65, t_steps*32]
def _wb_cols(t_steps):
    return OFF_XT + t_steps * BS

_CACHED = {}


def _x2g(Wp):
    """Scale the g-gate columns (m*256+192 .. +256) of a permuted [K, 1024]
    block by 2 so tanh(z_g) can be recovered from sigmoid(2*z_g)."""
    Wp = Wp.copy()
    for m in range(4):
        Wp[:, m * 256 + 192 : m * 256 + 256] *= 2.0
    return Wp


def _perm_cols(Wt):
    """Permute gate columns of [K, 1024] (col j = gate_orig*256 + h) into
    col = m*256 + gate_new*64 + hh*32 + hl, where h = hh*128 + m*32 + hl."""
    K = Wt.shape[0]
    W = Wt.reshape(K, 4, 256)[:, GATE_PERM, :]      # [K, gate, h]
    W = W.reshape(K, 4, 2, 4, 32)                    # [K, gate, hh, m, hl]
    W = W.transpose(0, 3, 1, 2, 4)                   # [K, m, gate, hh, hl]
    return np.ascontiguousarray(W.reshape(K, 1024), dtype=np.float32)


def _build_bass(t_steps=T):
    import concourse.mybir as mybir
    import concourse.tile as tile
    from concourse import bacc
    from contextlib import ExitStack

    f32 = mybir.dt.float32
    f16 = mybir.dt.float16
    AF = mybir.ActivationFunctionType

    nc = bacc.Bacc("TRN2", target_bir_lowering=False)

    wb_cols = _wb_cols(t_steps)
    wb_d = nc.dram_tensor("wb", (128, wb_cols), f16, kind="ExternalInput")
    y_d = nc.dram_tensor("y", (BS, O), f32, kind="ExternalOutput")

    with tile.TileContext(nc) as tc, ExitStack() as ctx:
        const = ctx.enter_context(tc.tile_pool(name="const", bufs=1))
        state = ctx.enter_context(tc.tile_pool(name="state", bufs=1))
        work = ctx.enter_context(tc.tile_pool(name="work", bufs=4))
        hts = ctx.enter_context(tc.tile_pool(name="hts", bufs=4))
        psum = ctx.enter_context(tc.tile_pool(name="psum", bufs=3, space="PSUM"))

        wb = const.tile([128, wb_cols], f16)
        nc.sync.dma_start(wb[:, 0:OFF_XT], wb_d[:, 0:OFF_XT])
        nxt = (wb_cols - OFF_XT) // 4
        for ck in range(4):
            a = OFF_XT + ck * nxt
            b = OFF_XT + (ck + 1) * nxt if ck < 3 else wb_cols
            nc.sync.dma_start(wb[:, a:b], wb_d[:, a:b])

        def xt_ap(t):
            return wb[0:65, OFF_XT + BS * t : OFF_XT + BS * t + BS]

        def w0_ap(kc, m):
            return wb[:, OFF_W0 + 1024 * kc + 256 * m : OFF_W0 + 1024 * kc + 256 * m + 256]

        def w1_ap(kc, m):
            return wb[:, OFF_W1 + 1024 * kc + 256 * m : OFF_W1 + 1024 * kc + 256 * m + 256]

        def wx1_ap(kc, m):
            return wb[:, OFF_WX1 + 1024 * kc + 256 * m : OFF_WX1 + 1024 * kc + 256 * m + 256]

        def wx0_ap(m):
            return wb[0:65, OFF_WX0 + 256 * m : OFF_WX0 + 256 * m + 256]

        def b1_ap(m):
            return wb[0:1, OFF_B1 + 256 * m : OFF_B1 + 256 * m + 256]

        c0 = state.tile([128, 64], f16)
        c1 = state.tile([128, 64], f16)
        nc.vector.memset(c0[:], 0.0)
        nc.vector.memset(c1[:], 0.0)
        hT0 = hts.tile([128, 64], f16, tag="ht0")
        hT1 = hts.tile([128, 64], f16, tag="ht1")
        nc.vector.memset(hT0[:], 0.0)
        nc.vector.memset(hT1[:], 0.0)
        ones_t = const.tile([1, BS], f16)
        nc.vector.memset(ones_t[:], 1.0)
        ones_ap = ones_t[:]

        import os
        def elementwise(g, c, tagsuf):
            sg = work.tile([128, 256], f16, tag="sg" + tagsuf)
            nc.scalar.activation(sg[:, 0:192], g[:, 0:192], AF.Sigmoid)
            nc.scalar.activation(sg[:, 192:256], g[:, 192:256], AF.Tanh)
            nc.vector.tensor_mul(c[:], sg[:, 64:128], c[:])
            m1 = work.tile([128, 64], f16, tag="m1" + tagsuf)
            nc.vector.tensor_mul(m1[:], sg[:, 0:64], sg[:, 192:256])
            nc.vector.tensor_add(c[:], c[:], m1[:])
            th = work.tile([128, 64], f16, tag="th" + tagsuf)
            nc.scalar.activation(th[:], c[:], AF.Tanh)
            h = work.tile([128, 64], f16, tag="h" + tagsuf)
            nc.vector.tensor_mul(h[:], sg[:, 128:192], th[:])
            hT = hts.tile([128, 64], f16, tag="ht" + tagsuf)
            nc.vector.transpose(hT[:], h[:])
            return hT

        def step0(t, hT0_prev):
            g = psum.tile([128, 256], f32, tag="g0")
            for m in range(4):
                nc.tensor.matmul(
                    g[32 * m : 32 * m + 32, :], xt_ap(t), wx0_ap(m),
                    start=True, stop=False, tile_position=(0, 32 * m), skip_group_check=True,
                )
            for kc in range(2):
                for m in range(4):
                    nc.tensor.matmul(
                        g[32 * m : 32 * m + 32, :],
                        hT0_prev[:, 32 * kc : 32 * kc + 32], w0_ap(kc, m),
                        start=False, stop=(kc == 1), tile_position=(0, 32 * m), skip_group_check=True,
                    )
            return elementwise(g, c0, "0")

        def step1(hT0_t, hT1_prev):
            g = psum.tile([128, 256], f32, tag="g1")
            for m in range(4):
                nc.tensor.matmul(
                    g[32 * m : 32 * m + 32, :], ones_ap, b1_ap(m),
                    start=True, stop=False, tile_position=(0, 32 * m), skip_group_check=True,
                )
            for si, (src, w_ap) in enumerate(((hT0_t, wx1_ap), (hT1_prev, w1_ap))):
                last_src = w_ap is w1_ap
                for kc in range(2):
                    for m in range(4):
                        nc.tensor.matmul(
                            g[32 * m : 32 * m + 32, :],
                            src[:, 32 * kc : 32 * kc + 32], w_ap(kc, m),
                            start=False,
                            stop=(last_src and kc == 1),
                            tile_position=(0, 32 * m), skip_group_check=True,
                        )
            return elementwise(g, c1, "1")

        hT0_hist = [hT0]
        for t in range(t_steps):
            hT0_new = step0(t, hT0_hist[-1])
            hT0_hist.append(hT0_new)
            # layer 1 lags by one step so the two chains overlap
            if t >= 1:
                hT1 = step1(hT0_hist[-2], hT1)
            if len(hT0_hist) > 3:
                hT0_hist.pop(0)
        hT1 = step1(hT0_hist[-1], hT1)

        yp = psum.tile([BS, O], f32, tag="yh", bufs=1)
        nc.tensor.matmul(yp[:], ones_ap, wb[0:1, OFF_BF : OFF_BF + O], start=True, stop=False)
        nc.tensor.matmul(yp[:], hT1[:, 0:32], wb[:, OFF_WF : OFF_WF + O], start=False, stop=False)
        nc.tensor.matmul(yp[:], hT1[:, 32:64], wb[:, OFF_WF + O : OFF_WF + 2 * O], start=False, stop=True)
        y_sb = work.tile([BS, O], f32, tag="y")
        nc.vector.tensor_copy(y_sb[:], yp[:])
        nc.sync.dma_start(y_d[:], y_sb[:])

    return nc


def _prep_inputs(x, Wih0, Whh0, bih0, bhh0, Wih1, Whh1, bih1, bhh1, W1, b1, W2, b2,
                 t_steps=T):
    x = np.asarray(x, dtype=np.float32)[:, :t_steps, :]
    wb = np.zeros((128, _wb_cols(t_steps)), np.float16)
    wb[:, OFF_W0 : OFF_W0 + 2048] = (_perm_cols(
        np.asarray(Whh0, np.float32).T)).reshape(2, 128, 1024).transpose(1, 0, 2).reshape(128, 2048)
    wb[:, OFF_W1 : OFF_W1 + 2048] = (_perm_cols(
        np.asarray(Whh1, np.float32).T)).reshape(2, 128, 1024).transpose(1, 0, 2).reshape(128, 2048)
    wb[:, OFF_WX1 : OFF_WX1 + 2048] = (_perm_cols(
        np.asarray(Wih1, np.float32).T)).reshape(2, 128, 1024).transpose(1, 0, 2).reshape(128, 2048)
    wb[0:64, OFF_WX0 : OFF_WX0 + 1024] = (_perm_cols(np.asarray(Wih0, np.float32).T))
    wb[64, OFF_WX0 : OFF_WX0 + 1024] = _perm_cols(
        (np.asarray(bih0, np.float32) + np.asarray(bhh0, np.float32))[None, :])[0]
    wb[0, OFF_B1 : OFF_B1 + 1024] = _perm_cols(
        (np.asarray(bih1, np.float32) + np.asarray(bhh1, np.float32))[None, :])[0]
    Wf = (np.asarray(W1, np.float32).T @ np.asarray(W2, np.float32).T).astype(np.float32)
    wb[:, OFF_WF : OFF_WF + 2 * O] = Wf.reshape(2, 128, O).transpose(1, 0, 2).reshape(128, 2 * O)
    wb[0, OFF_BF : OFF_BF + O] = (
        np.asarray(b1, np.float32) @ np.asarray(W2, np.float32).T + np.asarray(b2, np.float32))

    in_maps = []
    for c in range(NCORES):
        xc = x[c * BS : (c + 1) * BS]                       # [BS, t, I]
        xt = xc.transpose(2, 1, 0).reshape(I, t_steps * BS) # [I, t*BS]
        wbc = wb.copy()
        wbc[0:64, OFF_XT:] = xt
        wbc[64, OFF_XT:] = 1.0
        in_maps.append(dict(wb=wbc))
    return in_maps


def run(t_steps=T, trace=False, **inputs):
    from concourse.bass_utils import run_bass_kernel_spmd

    key = t_steps
    if key not in _CACHED:
        nc_new = _build_bass(t_steps)
        # finalize BEFORE handing to the PJRT path: the bass_exec lowering
        # otherwise finalizes with the partition-id register preamble in a
        # state that miscompiles (walrus "Reg has not been allocated yet")
        nc_new.finalize()
        _CACHED[key] = nc_new
    nc = _CACHED[key]
    in_maps = _prep_inputs(**inputs, t_steps=t_steps)
    res = None
    for attempt in range(4):
        try:
            res = run_bass_kernel_spmd(nc, in_maps, core_ids=list(range(NCORES)),
                                       trace=trace)
            break
        except Exception as e:  # flaky parallel-birverifier race in neuronx-cc
            if attempt == 3:
                raise
            print(f"run attempt {attempt} failed ({type(e).__name__}); retrying")
    assert res is not None
    y = np.concatenate([r["y"] for r in res.results], axis=0)
    return y, res


def kernel(**inputs):
    y, _ = run(t_steps=T, trace=False, **inputs)
    return y



# revision 19
# speedup vs baseline: 1.1384x; 1.0093x over previous
"""Trainium2 Bass kernel for a 2-layer LSTM (B=256, T=512, I=64, H=256) + linear head.

Strategy (hardcoded, self-contained):
  - Data-parallel over batch across 8 NeuronCores (32 batch elems per core).
  - Per core, both LSTM layers run step-by-step in a feature-blocked layout:
      gate PSUM tile [128=(hblk4, b32), 256=(gate4, hh2, hl32)]
    produced by col-group-packed matmuls (tile_position=(0, 32*m)) that share
    the small transposed-state stationary hT [k, 32].
  - Input projection x@Wih.T and all biases ride the same PSUM accumulation
    (augmented ones-row trick), so there is no separate projection pass.
  - Elementwise gate math runs at full 128 partitions; a single DVE 32x32
    block-transpose per layer-step turns h back into the next step's
    stationary hT.
  - The two output linear layers have no nonlinearity between them and are
    folded host-side into a single [256, 4] matmul + bias.
  - All weights ship as ONE packed DRAM blob (fp16) split into a weights DMA
    plus 4 x-chunk DMAs so step 0 starts before the full input lands.
  - All matmul operands are fp16 (1 PE cycle/row vs 4 for fp32; fp32 PSUM
    accumulation); sigmoid/tanh outputs, gate intermediates, and the cell
    state are fp16 for DVE 2x mode.  The f*c update is emitted before
    i*tanh(g) so it overlaps the tanh-g activation on the critical chain.
    Measured 1.57 ms vs 2.11 ms all-fp32, rel err 7.1e-4.
"""

import numpy as np

B, T, I, H, O = 256, 512, 64, 256, 4
NCORES = 8
BS = B // NCORES  # 32

# reference gate order is (i, f, g, o); we reorder to (i, f, o, g) so that the
# sigmoid gates are contiguous (cols 0:192) and tanh(g) is cols 192:256.
GATE_PERM = [0, 1, 3, 2]

# weight blob column offsets (fp32 elements, [128, WB_COLS])
OFF_W0 = 0        # Whh0 perm  [128, 2*1024]
OFF_W1 = 2048     # Whh1 perm  [128, 2*1024]
OFF_WX1 = 4096    # Wih1 perm  [128, 2*1024]
OFF_WX0 = 6144    # Wih0 perm + bias row, rows 0:65, [65, 1024]
OFF_B1 = 7168     # bias1 row, row 0, [1, 1024]
OFF_WF = 8192     # folded head weight [128, 2*4]
OFF_BF = 8200     # folded head bias, row 0, [1, 4]
OFF_XT = 8224     # x transposed + ones row, rows 0:65, [65, t_steps*32]
def _wb_cols(t_steps):
    return OFF_XT + t_steps * BS

_CACHED = {}


def _x2g(Wp):
    """Scale the g-gate columns (m*256+192 .. +256) of a permuted [K, 1024]
    block by 2 so tanh(z_g) can be recovered from sigmoid(2*z_g)."""
    Wp = Wp.copy()
    for m in range(4):
        Wp[:, m * 256 + 192 : m * 256 + 256] *= 2.0
    return Wp


def _perm_cols(Wt):
    """Permute gate columns of [K, 1024] (col j = gate_orig*256 + h) into
    col = m*256 + gate_new*64 + hh*32 + hl, where h = hh*128 + m*32 + hl."""
    K = Wt.shape[0]
    W = Wt.reshape(K, 4, 256)[:, GATE_PERM, :]      # [K, gate, h]
    W = W.reshape(K, 4, 2, 4, 32)                    # [K, gate, hh, m, hl]
    W = W.transpose(0, 3, 1, 2, 4)                   # [K, m, gate, hh, hl]
    return np.ascontiguousarray(W.reshape(K, 1024), dtype=np.float32)


def _build_bass(t_steps=T):
    import concourse.mybir as mybir
    import concourse.tile as tile
    from concourse import bacc
    from contextlib import ExitStack

    f32 = mybir.dt.float32
    f16 = mybir.dt.float16
    AF = mybir.ActivationFunctionType

    nc = bacc.Bacc("TRN2", target_bir_lowering=False)

    wb_cols = _wb_cols(t_steps)
    wb_d = nc.dram_tensor("wb", (128, wb_cols), f16, kind="ExternalInput")
    y_d = nc.dram_tensor("y", (BS, O), f32, kind="ExternalOutput")

    with tile.TileContext(nc) as tc, ExitStack() as ctx:
        const = ctx.enter_context(tc.tile_pool(name="const", bufs=1))
        state = ctx.enter_context(tc.tile_pool(name="state", bufs=1))
        work = ctx.enter_context(tc.tile_pool(name="work", bufs=4))
        hts = ctx.enter_context(tc.tile_pool(name="hts", bufs=4))
        psum = ctx.enter_context(tc.tile_pool(name="psum", bufs=3, space="PSUM"))

        wb = const.tile([128, wb_cols], f16)
        nc.sync.dma_start(wb[:, 0:OFF_XT], wb_d[:, 0:OFF_XT])
        nxt = (wb_cols - OFF_XT) // 4
        for ck in range(4):
            a = OFF_XT + ck * nxt
            b = OFF_XT + (ck + 1) * nxt if ck < 3 else wb_cols
            nc.sync.dma_start(wb[:, a:b], wb_d[:, a:b])

        def xt_ap(t):
            return wb[0:65, OFF_XT + BS * t : OFF_XT + BS * t + BS]

        def w0_ap(kc, m):
            return wb[:, OFF_W0 + 1024 * kc + 256 * m : OFF_W0 + 1024 * kc + 256 * m + 256]

        def w1_ap(kc, m):
            return wb[:, OFF_W1 + 1024 * kc + 256 * m : OFF_W1 + 1024 * kc + 256 * m + 256]

        def wx1_ap(kc, m):
            return wb[:, OFF_WX1 + 1024 * kc + 256 * m : OFF_WX1 + 1024 * kc + 256 * m + 256]

        def wx0_ap(m):
            return wb[0:65, OFF_WX0 + 256 * m : OFF_WX0 + 256 * m + 256]

        def b1_ap(m):
            return wb[0:1, OFF_B1 + 256 * m : OFF_B1 + 256 * m + 256]

        c0 = state.tile([128, 64], f16)
        c1 = state.tile([128, 64], f16)
        nc.vector.memset(c0[:], 0.0)
        nc.vector.memset(c1[:], 0.0)
        hT0 = hts.tile([128, 64], f16, tag="ht0")
        hT1 = hts.tile([128, 64], f16, tag="ht1")
        nc.vector.memset(hT0[:], 0.0)
        nc.vector.memset(hT1[:], 0.0)
        ones_t = const.tile([1, BS], f16)
        nc.vector.memset(ones_t[:], 1.0)
        ones_ap = ones_t[:]

        import os
        def elementwise(g, c, tagsuf):
            sg = work.tile([128, 256], f16, tag="sg" + tagsuf)
            nc.scalar.activation(sg[:, 0:192], g[:, 0:192], AF.Sigmoid)
            nc.scalar.activation(sg[:, 192:256], g[:, 192:256], AF.Tanh)
            nc.vector.tensor_mul(c[:], sg[:, 64:128], c[:])
            m1 = work.tile([128, 64], f16, tag="m1" + tagsuf)
            nc.vector.tensor_mul(m1[:], sg[:, 0:64], sg[:, 192:256])
            nc.vector.tensor_add(c[:], c[:], m1[:])
            th = work.tile([128, 64], f16, tag="th" + tagsuf)
            nc.scalar.activation(th[:], c[:], AF.Tanh)
            h = work.tile([128, 64], f16, tag="h" + tagsuf)
            nc.vector.tensor_mul(h[:], sg[:, 128:192], th[:])
            hT = hts.tile([128, 64], f16, tag="ht" + tagsuf)
            # split so the next step's kc0 stationary (cols 0:32) unblocks
            # early; the second half hides under the kc0 matmul stream
            nc.vector.transpose(hT[:, 0:32], h[:, 0:32])
            nc.vector.transpose(hT[:, 32:64], h[:, 32:64])
            return hT

        def step0(t, hT0_prev):
            g = psum.tile([128, 256], f32, tag="g0")
            for m in range(4):
                nc.tensor.matmul(
                    g[32 * m : 32 * m + 32, :], xt_ap(t), wx0_ap(m),
                    start=True, stop=False, tile_position=(0, 32 * m), skip_group_check=True,
                )
            for kc in range(2):
                for m in range(4):
                    nc.tensor.matmul(
                        g[32 * m : 32 * m + 32, :],
                        hT0_prev[:, 32 * kc : 32 * kc + 32], w0_ap(kc, m),
                        start=False, stop=(kc == 1), tile_position=(0, 32 * m), skip_group_check=True,
                    )
            return elementwise(g, c0, "0")

        def step1(hT0_t, hT1_prev):
            g = psum.tile([128, 256], f32, tag="g1")
            for m in range(4):
                nc.tensor.matmul(
                    g[32 * m : 32 * m + 32, :], ones_ap, b1_ap(m),
                    start=True, stop=False, tile_position=(0, 32 * m), skip_group_check=True,
                )
            for si, (src, w_ap) in enumerate(((hT0_t, wx1_ap), (hT1_prev, w1_ap))):
                last_src = w_ap is w1_ap
                for kc in range(2):
                    for m in range(4):
                        nc.tensor.matmul(
                            g[32 * m : 32 * m + 32, :],
                            src[:, 32 * kc : 32 * kc + 32], w_ap(kc, m),
                            start=False,
                            stop=(last_src and kc == 1),
                            tile_position=(0, 32 * m), skip_group_check=True,
                        )
            return elementwise(g, c1, "1")

        hT0_hist = [hT0]
        for t in range(t_steps):
            hT0_new = step0(t, hT0_hist[-1])
            hT0_hist.append(hT0_new)
            # layer 1 lags by one step so the two chains overlap
            if t >= 1:
                hT1 = step1(hT0_hist[-2], hT1)
            if len(hT0_hist) > 3:
                hT0_hist.pop(0)
        hT1 = step1(hT0_hist[-1], hT1)

        yp = psum.tile([BS, O], f32, tag="yh", bufs=1)
        nc.tensor.matmul(yp[:], ones_ap, wb[0:1, OFF_BF : OFF_BF + O], start=True, stop=False)
        nc.tensor.matmul(yp[:], hT1[:, 0:32], wb[:, OFF_WF : OFF_WF + O], start=False, stop=False)
        nc.tensor.matmul(yp[:], hT1[:, 32:64], wb[:, OFF_WF + O : OFF_WF + 2 * O], start=False, stop=True)
        y_sb = work.tile([BS, O], f32, tag="y")
        nc.vector.tensor_copy(y_sb[:], yp[:])
        nc.sync.dma_start(y_d[:], y_sb[:])

    return nc


def _prep_inputs(x, Wih0, Whh0, bih0, bhh0, Wih1, Whh1, bih1, bhh1, W1, b1, W2, b2,
                 t_steps=T):
    x = np.asarray(x, dtype=np.float32)[:, :t_steps, :]
    wb = np.zeros((128, _wb_cols(t_steps)), np.float16)
    wb[:, OFF_W0 : OFF_W0 + 2048] = (_perm_cols(
        np.asarray(Whh0, np.float32).T)).reshape(2, 128, 1024).transpose(1, 0, 2).reshape(128, 2048)
    wb[:, OFF_W1 : OFF_W1 + 2048] = (_perm_cols(
        np.asarray(Whh1, np.float32).T)).reshape(2, 128, 1024).transpose(1, 0, 2).reshape(128, 2048)
    wb[:, OFF_WX1 : OFF_WX1 + 2048] = (_perm_cols(
        np.asarray(Wih1, np.float32).T)).reshape(2, 128, 1024).transpose(1, 0, 2).reshape(128, 2048)
    wb[0:64, OFF_WX0 : OFF_WX0 + 1024] = (_perm_cols(np.asarray(Wih0, np.float32).T))
    wb[64, OFF_WX0 : OFF_WX0 + 1024] = _perm_cols(
        (np.asarray(bih0, np.float32) + np.asarray(bhh0, np.float32))[None, :])[0]
    wb[0, OFF_B1 : OFF_B1 + 1024] = _perm_cols(
        (np.asarray(bih1, np.float32) + np.asarray(bhh1, np.float32))[None, :])[0]
    Wf = (np.asarray(W1, np.float32).T @ np.asarray(W2, np.float32).T).astype(np.float32)
    wb[:, OFF_WF : OFF_WF + 2 * O] = Wf.reshape(2, 128, O).transpose(1, 0, 2).reshape(128, 2 * O)
    wb[0, OFF_BF : OFF_BF + O] = (
        np.asarray(b1, np.float32) @ np.asarray(W2, np.float32).T + np.asarray(b2, np.float32))

    in_maps = []
    for c in range(NCORES):
        xc = x[c * BS : (c + 1) * BS]                       # [BS, t, I]
        xt = xc.transpose(2, 1, 0).reshape(I, t_steps * BS) # [I, t*BS]
        wbc = wb.copy()
        wbc[0:64, OFF_XT:] = xt
        wbc[64, OFF_XT:] = 1.0
        in_maps.append(dict(wb=wbc))
    return in_maps


def run(t_steps=T, trace=False, **inputs):
    from concourse.bass_utils import run_bass_kernel_spmd

    key = t_steps
    if key not in _CACHED:
        nc_new = _build_bass(t_steps)
        # finalize BEFORE handing to the PJRT path: the bass_exec lowering
        # otherwise finalizes with the partition-id register preamble in a
        # state that miscompiles (walrus "Reg has not been allocated yet")
        nc_new.finalize()
        _CACHED[key] = nc_new
    nc = _CACHED[key]
    in_maps = _prep_inputs(**inputs, t_steps=t_steps)
    res = None
    for attempt in range(4):
        try:
            res = run_bass_kernel_spmd(nc, in_maps, core_ids=list(range(NCORES)),
                                       trace=trace)
            break
        except Exception as e:  # flaky parallel-birverifier race in neuronx-cc
            if attempt == 3:
                raise
            print(f"run attempt {attempt} failed ({type(e).__name__}); retrying")
    assert res is not None
    y = np.concatenate([r["y"] for r in res.results], axis=0)
    return y, res


def kernel(**inputs):
    y, _ = run(t_steps=T, trace=False, **inputs)
    return y



# revision 20
# speedup vs baseline: 1.1604x; 1.0193x over previous
"""Trainium2 Bass kernel for a 2-layer LSTM (B=256, T=512, I=64, H=256) + linear head.

Strategy (hardcoded, self-contained):
  - Data-parallel over batch across 8 NeuronCores (32 batch elems per core).
  - Per core, both LSTM layers run step-by-step in a feature-blocked layout:
      gate PSUM tile [128=(hblk4, b32), 256=(gate4, hh2, hl32)]
    produced by col-group-packed matmuls (tile_position=(0, 32*m)) that share
    the small transposed-state stationary hT [k, 32].
  - Input projection x@Wih.T and all biases ride the same PSUM accumulation
    (augmented ones-row trick), so there is no separate projection pass.
  - Elementwise gate math runs at full 128 partitions; a single DVE 32x32
    block-transpose per layer-step turns h back into the next step's
    stationary hT.
  - The two output linear layers have no nonlinearity between them and are
    folded host-side into a single [256, 4] matmul + bias.
  - All weights ship as ONE packed DRAM blob (fp16) split into a weights DMA
    plus 4 x-chunk DMAs so step 0 starts before the full input lands.
  - All matmul operands are fp16 (1 PE cycle/row vs 4 for fp32; fp32 PSUM
    accumulation); sigmoid/tanh outputs, gate intermediates, and the cell
    state are fp16 for DVE 2x mode.  The f*c update is emitted before
    i*tanh(g) so it overlaps the tanh-g activation on the critical chain.
    Measured 1.57 ms vs 2.11 ms all-fp32, rel err 7.1e-4.
"""

import numpy as np

B, T, I, H, O = 256, 512, 64, 256, 4
NCORES = 8
BS = B // NCORES  # 32

# reference gate order is (i, f, g, o); we reorder to (i, f, o, g) so that the
# sigmoid gates are contiguous (cols 0:192) and tanh(g) is cols 192:256.
GATE_PERM = [0, 1, 3, 2]

# weight blob column offsets (fp32 elements, [128, WB_COLS])
OFF_W0 = 0        # Whh0 perm  [128, 2*1024]
OFF_W1 = 2048     # Whh1 perm  [128, 2*1024]
OFF_WX1 = 4096    # Wih1 perm  [128, 2*1024]
OFF_WX0 = 6144    # Wih0 perm + bias row, rows 0:65, [65, 1024]
OFF_B1 = 7168     # bias1 row, row 0, [1, 1024]
OFF_WF = 8192     # folded head weight [128, 2*4]
OFF_BF = 8200     # folded head bias, row 0, [1, 4]
OFF_XT = 8224     # x transposed + ones row, rows 0:65, [65, t_steps*32]
def _wb_cols(t_steps):
    return OFF_XT + t_steps * BS

_CACHED = {}


def _x2g(Wp):
    """Scale the g-gate columns (m*256+192 .. +256) of a permuted [K, 1024]
    block by 2 so tanh(z_g) can be recovered from sigmoid(2*z_g)."""
    Wp = Wp.copy()
    for m in range(4):
        Wp[:, m * 256 + 192 : m * 256 + 256] *= 2.0
    return Wp


def _perm_cols(Wt):
    """Permute gate columns of [K, 1024] (col j = gate_orig*256 + h) into
    col = m*256 + gate_new*64 + hh*32 + hl, where h = hh*128 + m*32 + hl."""
    K = Wt.shape[0]
    W = Wt.reshape(K, 4, 256)[:, GATE_PERM, :]      # [K, gate, h]
    W = W.reshape(K, 4, 2, 4, 32)                    # [K, gate, hh, m, hl]
    W = W.transpose(0, 3, 1, 2, 4)                   # [K, m, gate, hh, hl]
    return np.ascontiguousarray(W.reshape(K, 1024), dtype=np.float32)


def _build_bass(t_steps=T):
    import concourse.mybir as mybir
    import concourse.tile as tile
    from concourse import bacc
    from contextlib import ExitStack

    f32 = mybir.dt.float32
    f16 = mybir.dt.float16
    AF = mybir.ActivationFunctionType

    nc = bacc.Bacc("TRN2", target_bir_lowering=False)

    wb_cols = _wb_cols(t_steps)
    wb_d = nc.dram_tensor("wb", (128, wb_cols), f16, kind="ExternalInput")
    y_d = nc.dram_tensor("y", (BS, O), f32, kind="ExternalOutput")

    with tile.TileContext(nc) as tc, ExitStack() as ctx:
        const = ctx.enter_context(tc.tile_pool(name="const", bufs=1))
        state = ctx.enter_context(tc.tile_pool(name="state", bufs=1))
        work = ctx.enter_context(tc.tile_pool(name="work", bufs=4))
        hts = ctx.enter_context(tc.tile_pool(name="hts", bufs=4))
        psum = ctx.enter_context(tc.tile_pool(name="psum", bufs=3, space="PSUM"))

        wb = const.tile([128, wb_cols], f16)
        nc.sync.dma_start(wb[:, 0:OFF_XT], wb_d[:, 0:OFF_XT])
        nxt = (wb_cols - OFF_XT) // 4
        for ck in range(4):
            a = OFF_XT + ck * nxt
            b = OFF_XT + (ck + 1) * nxt if ck < 3 else wb_cols
            nc.sync.dma_start(wb[:, a:b], wb_d[:, a:b])

        def xt_ap(t):
            return wb[0:65, OFF_XT + BS * t : OFF_XT + BS * t + BS]

        def w0_ap(kc, m):
            return wb[:, OFF_W0 + 1024 * kc + 256 * m : OFF_W0 + 1024 * kc + 256 * m + 256]

        def w1_ap(kc, m):
            return wb[:, OFF_W1 + 1024 * kc + 256 * m : OFF_W1 + 1024 * kc + 256 * m + 256]

        def wx1_ap(kc, m):
            return wb[:, OFF_WX1 + 1024 * kc + 256 * m : OFF_WX1 + 1024 * kc + 256 * m + 256]

        def wx0_ap(m):
            return wb[0:65, OFF_WX0 + 256 * m : OFF_WX0 + 256 * m + 256]

        def b1_ap(m):
            return wb[0:1, OFF_B1 + 256 * m : OFF_B1 + 256 * m + 256]

        c0 = state.tile([128, 64], f16)
        c1 = state.tile([128, 64], f16)
        nc.vector.memset(c0[:], 0.0)
        nc.vector.memset(c1[:], 0.0)
        hT0 = hts.tile([128, 64], f16, tag="ht0")
        hT1 = hts.tile([128, 64], f16, tag="ht1")
        nc.vector.memset(hT0[:], 0.0)
        nc.vector.memset(hT1[:], 0.0)
        ones_t = const.tile([1, BS], f16)
        nc.vector.memset(ones_t[:], 1.0)
        ones_ap = ones_t[:]

        import os
        def elementwise(g, c, tagsuf):
            sg = work.tile([128, 256], f16, tag="sg" + tagsuf)
            # sigma over i,f only -- the o-gate sigmoid is deferred into the
            # ACT idle window during the DVE cell update (h needs it only
            # after tanh(c)), shortening the sigma->tanh-g->m1 chain
            nc.scalar.activation(sg[:, 0:128], g[:, 0:128], AF.Sigmoid)
            nc.scalar.activation(sg[:, 192:256], g[:, 192:256], AF.Tanh)
            nc.vector.tensor_mul(c[:], sg[:, 64:128], c[:])
            m1 = work.tile([128, 64], f16, tag="m1" + tagsuf)
            nc.vector.tensor_mul(m1[:], sg[:, 0:64], sg[:, 192:256])
            nc.vector.tensor_add(c[:], c[:], m1[:])
            nc.scalar.activation(sg[:, 128:192], g[:, 128:192], AF.Sigmoid)
            th = work.tile([128, 64], f16, tag="th" + tagsuf)
            nc.scalar.activation(th[:], c[:], AF.Tanh)
            h = work.tile([128, 64], f16, tag="h" + tagsuf)
            nc.vector.tensor_mul(h[:], sg[:, 128:192], th[:])
            hT = hts.tile([128, 64], f16, tag="ht" + tagsuf)
            # split so the next step's kc0 stationary (cols 0:32) unblocks
            # early; the second half hides under the kc0 matmul stream
            nc.vector.transpose(hT[:, 0:32], h[:, 0:32])
            nc.vector.transpose(hT[:, 32:64], h[:, 32:64])
            return hT

        def step0(t, hT0_prev):
            g = psum.tile([128, 256], f32, tag="g0")
            for m in range(4):
                nc.tensor.matmul(
                    g[32 * m : 32 * m + 32, :], xt_ap(t), wx0_ap(m),
                    start=True, stop=False, tile_position=(0, 32 * m), skip_group_check=True,
                )
            for kc in range(2):
                for m in range(4):
                    nc.tensor.matmul(
                        g[32 * m : 32 * m + 32, :],
                        hT0_prev[:, 32 * kc : 32 * kc + 32], w0_ap(kc, m),
                        start=False, stop=(kc == 1), tile_position=(0, 32 * m), skip_group_check=True,
                    )
            return elementwise(g, c0, "0")

        def step1(hT0_t, hT1_prev):
            g = psum.tile([128, 256], f32, tag="g1")
            for m in range(4):
                nc.tensor.matmul(
                    g[32 * m : 32 * m + 32, :], ones_ap, b1_ap(m),
                    start=True, stop=False, tile_position=(0, 32 * m), skip_group_check=True,
                )
            for si, (src, w_ap) in enumerate(((hT0_t, wx1_ap), (hT1_prev, w1_ap))):
                last_src = w_ap is w1_ap
                for kc in range(2):
                    for m in range(4):
                        nc.tensor.matmul(
                            g[32 * m : 32 * m + 32, :],
                            src[:, 32 * kc : 32 * kc + 32], w_ap(kc, m),
                            start=False,
                            stop=(last_src and kc == 1),
                            tile_position=(0, 32 * m), skip_group_check=True,
                        )
            return elementwise(g, c1, "1")

        hT0_hist = [hT0]
        for t in range(t_steps):
            hT0_new = step0(t, hT0_hist[-1])
            hT0_hist.append(hT0_new)
            # layer 1 lags by one step so the two chains overlap
            if t >= 1:
                hT1 = step1(hT0_hist[-2], hT1)
            if len(hT0_hist) > 3:
                hT0_hist.pop(0)
        hT1 = step1(hT0_hist[-1], hT1)

        yp = psum.tile([BS, O], f32, tag="yh", bufs=1)
        nc.tensor.matmul(yp[:], ones_ap, wb[0:1, OFF_BF : OFF_BF + O], start=True, stop=False)
        nc.tensor.matmul(yp[:], hT1[:, 0:32], wb[:, OFF_WF : OFF_WF + O], start=False, stop=False)
        nc.tensor.matmul(yp[:], hT1[:, 32:64], wb[:, OFF_WF + O : OFF_WF + 2 * O], start=False, stop=True)
        y_sb = work.tile([BS, O], f32, tag="y")
        nc.vector.tensor_copy(y_sb[:], yp[:])
        nc.sync.dma_start(y_d[:], y_sb[:])

    return nc


def _prep_inputs(x, Wih0, Whh0, bih0, bhh0, Wih1, Whh1, bih1, bhh1, W1, b1, W2, b2,
                 t_steps=T):
    x = np.asarray(x, dtype=np.float32)[:, :t_steps, :]
    wb = np.zeros((128, _wb_cols(t_steps)), np.float16)
    wb[:, OFF_W0 : OFF_W0 + 2048] = (_perm_cols(
        np.asarray(Whh0, np.float32).T)).reshape(2, 128, 1024).transpose(1, 0, 2).reshape(128, 2048)
    wb[:, OFF_W1 : OFF_W1 + 2048] = (_perm_cols(
        np.asarray(Whh1, np.float32).T)).reshape(2, 128, 1024).transpose(1, 0, 2).reshape(128, 2048)
    wb[:, OFF_WX1 : OFF_WX1 + 2048] = (_perm_cols(
        np.asarray(Wih1, np.float32).T)).reshape(2, 128, 1024).transpose(1, 0, 2).reshape(128, 2048)
    wb[0:64, OFF_WX0 : OFF_WX0 + 1024] = (_perm_cols(np.asarray(Wih0, np.float32).T))
    wb[64, OFF_WX0 : OFF_WX0 + 1024] = _perm_cols(
        (np.asarray(bih0, np.float32) + np.asarray(bhh0, np.float32))[None, :])[0]
    wb[0, OFF_B1 : OFF_B1 + 1024] = _perm_cols(
        (np.asarray(bih1, np.float32) + np.asarray(bhh1, np.float32))[None, :])[0]
    Wf = (np.asarray(W1, np.float32).T @ np.asarray(W2, np.float32).T).astype(np.float32)
    wb[:, OFF_WF : OFF_WF + 2 * O] = Wf.reshape(2, 128, O).transpose(1, 0, 2).reshape(128, 2 * O)
    wb[0, OFF_BF : OFF_BF + O] = (
        np.asarray(b1, np.float32) @ np.asarray(W2, np.float32).T + np.asarray(b2, np.float32))

    in_maps = []
    for c in range(NCORES):
        xc = x[c * BS : (c + 1) * BS]                       # [BS, t, I]
        xt = xc.transpose(2, 1, 0).reshape(I, t_steps * BS) # [I, t*BS]
        wbc = wb.copy()
        wbc[0:64, OFF_XT:] = xt
        wbc[64, OFF_XT:] = 1.0
        in_maps.append(dict(wb=wbc))
    return in_maps


def run(t_steps=T, trace=False, **inputs):
    from concourse.bass_utils import run_bass_kernel_spmd

    key = t_steps
    if key not in _CACHED:
        nc_new = _build_bass(t_steps)
        # finalize BEFORE handing to the PJRT path: the bass_exec lowering
        # otherwise finalizes with the partition-id register preamble in a
        # state that miscompiles (walrus "Reg has not been allocated yet")
        nc_new.finalize()
        _CACHED[key] = nc_new
    nc = _CACHED[key]
    in_maps = _prep_inputs(**inputs, t_steps=t_steps)
    res = None
    for attempt in range(4):
        try:
            res = run_bass_kernel_spmd(nc, in_maps, core_ids=list(range(NCORES)),
                                       trace=trace)
            break
        except Exception as e:  # flaky parallel-birverifier race in neuronx-cc
            if attempt == 3:
                raise
            print(f"run attempt {attempt} failed ({type(e).__name__}); retrying")
    assert res is not None
    y = np.concatenate([r["y"] for r in res.results], axis=0)
    return y, res


def kernel(**inputs):
    y, _ = run(t_steps=T, trace=False, **inputs)
    return y

